# revision 4
# baseline (speedup 1.0000x reference)
"""Causal self-attention (GQA, rope) on 8 Trainium2 NeuronCores.

Sharding: tensor-parallel over the 4 kv-head groups x data-parallel over the
batch of 2.  Core c handles batch b = c // 4 and kv-group g = c % 4:

  - q/k/v projections for the group's 4 q-heads + 1 kv-head,
  - rope, causal flash-style attention (unnormalized softmax: e = exp(s),
    y = (e @ v) * (1 / (e @ 1)) -- safe here because scores are O(1)),
  - partial output projection out_partial = y_g @ wproj[:, cols_g].T.

The host sums the 4 group partials per batch element.

All matmuls run in bf16 with fp32 PSUM accumulation.  Activations are cast
host-side; x / weights are pre-transposed host-side so the contraction dim
lands on SBUF partitions without any on-device fp32 transposes.
"""

import numpy as np
import ml_dtypes

BF16 = ml_dtypes.bfloat16

T = 2048
C = 2048
HD = 128
N_KV = 4
N_REP = 4
O_G = N_REP * HD  # 512 q-dims per group
TC = 512  # t-chunk (psum bank width in fp32)
N_TC = T // TC  # 4
N_KT = C // 128  # 16 contraction tiles
SCALE = float(1.0 / np.sqrt(HD))

_compiled = None


def _build():
    import concourse.bacc as bacc
    import concourse.mybir as mybir
    import concourse.tile as tile
    from concourse.masks import make_identity

    f32 = mybir.dt.float32
    bf16 = mybir.dt.bfloat16

    nc = bacc.Bacc("TRN2", target_bir_lowering=False, debug=False)

    xT = nc.dram_tensor("xT", [C, T], bf16, kind="ExternalInput").ap()
    wqT = nc.dram_tensor("wqT", [C, O_G], bf16, kind="ExternalInput").ap()
    wkT = nc.dram_tensor("wkT", [C, HD], bf16, kind="ExternalInput").ap()
    wvT = nc.dram_tensor("wvT", [C, HD], bf16, kind="ExternalInput").ap()
    wpT = nc.dram_tensor("wpT", [O_G, C], bf16, kind="ExternalInput").ap()
    bq = nc.dram_tensor("bq", [1, O_G], bf16, kind="ExternalInput").ap()
    bk = nc.dram_tensor("bk", [1, HD], bf16, kind="ExternalInput").ap()
    bv = nc.dram_tensor("bv", [1, HD], bf16, kind="ExternalInput").ap()
    ropeA = nc.dram_tensor("ropeA", [HD, T], f32, kind="ExternalInput").ap()
    ropeB = nc.dram_tensor("ropeB", [HD, T], f32, kind="ExternalInput").ap()
    masks = nc.dram_tensor("masks", [128, 4, TC], bf16, kind="ExternalInput").ap()
    out = nc.dram_tensor("out", [T, C], f32, kind="ExternalOutput").ap()

    Exp = mybir.ActivationFunctionType.Exp
    Copy = mybir.ActivationFunctionType.Copy

    with tile.TileContext(nc) as tc:
        import contextlib

        with contextlib.ExitStack() as ctx:
            persist = ctx.enter_context(tc.tile_pool(name="persist", bufs=1))

            # ---- persistent SBUF tensors ----
            wpT_sb = persist.tile([128, N_REP, C], bf16)
            qT_sb = persist.tile([128, N_REP, T], bf16)
            kT_sb = persist.tile([128, T], bf16)
            v_sb = persist.tile([128, N_KT, HD], bf16)
            yT_sb = persist.tile([128, N_REP, T], bf16)
            masks_sb = persist.tile([128, 4, TC], bf16)
            ones512 = persist.tile([1, TC], bf16)
            onescol = persist.tile([128, 1], bf16)
            onesrow = persist.tile([1, 128], bf16)
            ident = persist.tile([128, 128], bf16)

            # ---- phase B (projections): scoped pools ----
            bctx = contextlib.ExitStack()
            bpool = bctx.enter_context(tc.tile_pool(name="phase_b", bufs=1))
            tmp_pool = bctx.enter_context(tc.tile_pool(name="rope_tmp", bufs=2))
            xT_sb = bpool.tile([128, N_KT, T], bf16)
            wqT_sb = bpool.tile([128, N_KT, O_G], bf16)
            wkT_sb = bpool.tile([128, N_KT, HD], bf16)
            wvT_sb = bpool.tile([128, N_KT, HD], bf16)
            ropeA_sb = bpool.tile([128, T], f32)
            ropeB_sb = bpool.tile([128, T], f32)
            bq_sb = bpool.tile([1, O_G], bf16)
            bk_sb = bpool.tile([1, HD], bf16)
            bv_sb = bpool.tile([1, HD], bf16)
            vT_sb = bpool.tile([128, T], bf16)

            nc.vector.memset(ones512[:], 1.0)
            nc.vector.memset(onescol[:], 1.0)
            nc.vector.memset(onesrow[:], 1.0)
            make_identity(nc, ident[:])

            nc.sync.dma_start(masks_sb[:], masks[:])
            nc.sync.dma_start(wpT_sb[:], wpT.rearrange("(h p) m -> p h m", p=128))
            nc.sync.dma_start(wqT_sb[:], wqT.rearrange("(kt p) o -> p kt o", p=128))
            nc.sync.dma_start(wkT_sb[:], wkT.rearrange("(kt p) o -> p kt o", p=128))
            nc.sync.dma_start(wvT_sb[:], wvT.rearrange("(kt p) o -> p kt o", p=128))
            nc.sync.dma_start(bq_sb[:], bq[:])
            nc.sync.dma_start(bk_sb[:], bk[:])
            nc.sync.dma_start(bv_sb[:], bv[:])
            nc.sync.dma_start(ropeA_sb[:], ropeA[:])
            nc.sync.dma_start(ropeB_sb[:], ropeB[:])
            # x: one DMA per contraction tile so matmuls can start early
            for kt in range(N_KT):
                nc.sync.dma_start(
                    xT_sb[:, kt, :],
                    xT.rearrange("(kt p) t -> p kt t", p=128)[:, kt, :],
                )

            def rope_epilogue(ps, dst, t0):
                """dst (bf16 SBUF) = rope(ps) using A/B tables; ps is fp32 psum
                [128, TC] holding a projected qT/kT chunk at positions t0:t0+TC."""
                A = ropeA_sb[:, t0 : t0 + TC]
                Bm = ropeB_sb[:, t0 : t0 + TC]
                tmp = tmp_pool.tile([128, TC], f32, tag="rope_tmp")
                tmp2 = tmp_pool.tile([128, TC], f32, tag="rope_tmp2")
                nc.vector.tensor_mul(tmp[0:64, :], ps[64:128, :], Bm[0:64, :])
                nc.vector.tensor_mul(tmp[64:128, :], ps[0:64, :], Bm[64:128, :])
                nc.vector.tensor_mul(tmp2[:], ps[:], A)
                nc.vector.tensor_add(dst, tmp2[:], tmp[:])

            # ---- phase B1: q projection (transposed) + rope ----
            with tc.tile_pool(name="qpsum", bufs=8, space="PSUM") as qpsum:
                for tci in range(N_TC):
                    t0 = tci * TC
                    for ot in range(N_REP):
                        ps = qpsum.tile([128, TC], f32)
                        for kt in range(N_KT):
                            nc.tensor.matmul(
                                ps[:],
                                lhsT=wqT_sb[:, kt, ot * 128 : (ot + 1) * 128],
                                rhs=xT_sb[:, kt, t0 : t0 + TC],
                                start=(kt == 0),
                                stop=False,
                            )
                        nc.tensor.matmul(
                            ps[:],
                            lhsT=bq_sb[:, ot * 128 : (ot + 1) * 128],
                            rhs=ones512[:],
                            start=False,
                            stop=True,
                        )
                        rope_epilogue(ps, qT_sb[:, ot, t0 : t0 + TC], t0)

            # ---- phase B2: k, v projections ----
            with (
                tc.tile_pool(name="kpsum", bufs=2, space="PSUM") as kpsum,
                tc.tile_pool(name="vpsum", bufs=2, space="PSUM") as vpsum,
                tc.tile_pool(name="tpsum", bufs=2, space="PSUM") as tpsum,
            ):
                for tci in range(N_TC):
                    t0 = tci * TC
                    psk = kpsum.tile([128, TC], f32)
                    for kt in range(N_KT):
                        nc.tensor.matmul(
                            psk[:],
                            lhsT=wkT_sb[:, kt, :],
                            rhs=xT_sb[:, kt, t0 : t0 + TC],
                            start=(kt == 0),
                            stop=False,
                        )
                    nc.tensor.matmul(
                        psk[:], lhsT=bk_sb[:], rhs=ones512[:], start=False, stop=True
                    )
                    rope_epilogue(psk, kT_sb[:, t0 : t0 + TC], t0)

                    psv = vpsum.tile([128, TC], f32)
                    for kt in range(N_KT):
                        nc.tensor.matmul(
                            psv[:],
                            lhsT=wvT_sb[:, kt, :],
                            rhs=xT_sb[:, kt, t0 : t0 + TC],
                            start=(kt == 0),
                            stop=False,
                        )
                    nc.tensor.matmul(
                        psv[:], lhsT=bv_sb[:], rhs=ones512[:], start=False, stop=True
                    )
                    nc.scalar.copy(vT_sb[:, t0 : t0 + TC], psv[:])

                # transpose vT -> v (natural [t, d] tiles) via PE
                for jt in range(N_KT):
                    pst = tpsum.tile([128, 128], bf16)
                    nc.tensor.transpose(
                        pst[:], vT_sb[:, jt * 128 : (jt + 1) * 128], ident[:]
                    )
                    nc.scalar.copy(v_sb[:, jt, :], pst[:])

            bctx.close()

            # ---- phases C (attention) + D (output projection) ----
            small = ctx.enter_context(tc.tile_pool(name="small", bufs=2))
            stage_pool = ctx.enter_context(tc.tile_pool(name="stage", bufs=3))
            epool = ctx.enter_context(tc.tile_pool(name="e", bufs=4))
            with (
                tc.tile_pool(name="spsum", bufs=3, space="PSUM") as spsum,
                tc.tile_pool(name="ypsum", bufs=2, space="PSUM") as ypsum,
                tc.tile_pool(name="dpsum", bufs=1, space="PSUM") as dpsum,
                tc.tile_pool(name="opsum", bufs=2, space="PSUM") as opsum,
            ):
                for ci in range(N_TC):
                    i0 = ci * TC
                    njt = 4 * (ci + 1)
                    for h in range(N_REP):
                        ps_y = ypsum.tile([128, TC], f32)
                        ps_den = dpsum.tile([1, TC], f32)
                        for jt in range(njt):
                            ps_s = spsum.tile([128, TC], f32, tag="s")
                            nc.tensor.matmul(
                                ps_s[:],
                                lhsT=kT_sb[:, jt * 128 : (jt + 1) * 128],
                                rhs=qT_sb[:, h, i0 : i0 + TC],
                                start=True,
                                stop=True,
                            )
                            e = epool.tile([128, TC], bf16)
                            nc.scalar.activation(e[:], ps_s[:], Exp, scale=SCALE)
                            r = jt - 4 * ci
                            if r >= 0:
                                nc.vector.tensor_mul(e[:], e[:], masks_sb[:, r, :])
                            nc.tensor.matmul(
                                ps_y[:],
                                lhsT=v_sb[:, jt, :],
                                rhs=e[:],
                                start=(jt == 0),
                                stop=(jt == njt - 1),
                            )
                            nc.tensor.matmul(
                                ps_den[:],
                                lhsT=onescol[:],
                                rhs=e[:],
                                start=(jt == 0),
                                stop=(jt == njt - 1),
                            )
                        rden = small.tile([1, TC], f32, tag="rden")
                        nc.vector.reciprocal(rden[:], ps_den[:])
                        rden_bf = small.tile([1, TC], bf16, tag="rden_bf")
                        nc.scalar.activation(rden_bf[:], rden[:], Copy)
                        ps_rb = spsum.tile([128, TC], f32, tag="s")
                        nc.tensor.matmul(
                            ps_rb[:],
                            lhsT=onesrow[:],
                            rhs=rden_bf[:],
                            start=True,
                            stop=True,
                        )
                        rb_sb = stage_pool.tile([128, TC], f32, tag="rb_stage")
                        nc.scalar.copy(rb_sb[:], ps_rb[:])
                        nc.vector.tensor_mul(
                            yT_sb[:, h, i0 : i0 + TC], ps_y[:], rb_sb[:]
                        )

                    # phase D for the 4 t-subtiles of this chunk
                    for ts_ in range(4):
                        t_idx = ci * 4 + ts_
                        for mc in range(N_TC):
                            ps_o = opsum.tile([128, TC], f32)
                            for h in range(N_REP):
                                nc.tensor.matmul(
                                    ps_o[:],
                                    lhsT=yT_sb[:, h, t_idx * 128 : (t_idx + 1) * 128],
                                    rhs=wpT_sb[:, h, mc * TC : (mc + 1) * TC],
                                    start=(h == 0),
                                    stop=(h == N_REP - 1),
                                )
                            o_sb = stage_pool.tile([128, TC], f32, tag="o_stage")
                            # split psum->sbuf copies across ACT and DVE
                            if mc % 2 == 0:
                                nc.scalar.copy(o_sb[:], ps_o[:])
                            else:
                                nc.vector.tensor_copy(o_sb[:], ps_o[:])
                            nc.sync.dma_start(
                                out[
                                    t_idx * 128 : (t_idx + 1) * 128,
                                    mc * TC : (mc + 1) * TC,
                                ],
                                o_sb[:],
                            )

    nc.compile()
    return nc


def _get_compiled():
    global _compiled
    if _compiled is None:
        _compiled = _build()
    return _compiled


def kernel(x, cos, sin, wq, bq, wk, bk, wv, bv, wproj):
    from concourse.bass_utils import run_bass_kernel_spmd

    nc = _get_compiled()

    x = np.asarray(x, np.float32)
    cosT = np.asarray(cos, np.float32)[0, :, 0, :].T  # (64, T)
    sinT = np.asarray(sin, np.float32)[0, :, 0, :].T
    ropeA = np.ascontiguousarray(np.concatenate([cosT, cosT], 0))  # (128, T)
    ropeB = np.ascontiguousarray(np.concatenate([-sinT, sinT], 0))

    jj = np.arange(128, dtype=np.int64)[:, None, None]
    rr = np.arange(4, dtype=np.int64)[None, :, None]
    ii = np.arange(TC, dtype=np.int64)[None, None, :]
    masks = ((128 * rr + jj) <= ii).astype(BF16)  # (128, 4, 512)

    xT = [np.ascontiguousarray(x[b].T).astype(BF16) for b in range(2)]

    in_maps = []
    for c in range(8):
        b, g = divmod(c, 4)
        in_maps.append(
            {
                "xT": xT[b],
                "wqT": np.ascontiguousarray(
                    wq[g * O_G : (g + 1) * O_G].T
                ).astype(BF16),
                "wkT": np.ascontiguousarray(
                    wk[g * HD : (g + 1) * HD].T
                ).astype(BF16),
                "wvT": np.ascontiguousarray(
                    wv[g * HD : (g + 1) * HD].T
                ).astype(BF16),
                "wpT": np.ascontiguousarray(
                    wproj[:, g * O_G : (g + 1) * O_G].T
                ).astype(BF16),
                "bq": bq[None, g * O_G : (g + 1) * O_G].astype(BF16),
                "bk": bk[None, g * HD : (g + 1) * HD].astype(BF16),
                "bv": bv[None, g * HD : (g + 1) * HD].astype(BF16),
                "ropeA": ropeA,
                "ropeB": ropeB,
                "masks": masks,
            }
        )

    res = run_bass_kernel_spmd(nc, in_maps, core_ids=list(range(8)))
    parts = [res.results[c]["out"] for c in range(8)]
    out = np.stack(
        [
            parts[0] + parts[1] + parts[2] + parts[3],
            parts[4] + parts[5] + parts[6] + parts[7],
        ]
    ).astype(np.float32)
    return out


# revision 5
# speedup vs baseline: 1.0732x; 1.0732x over previous
"""Causal self-attention (GQA, rope) on 8 Trainium2 NeuronCores.

Sharding: tensor-parallel over the 4 kv-head groups x data-parallel over the
batch of 2.  Core c handles batch b = c // 4 and kv-group g = c % 4:

  - q/k/v projections for the group's 4 q-heads + 1 kv-head,
  - rope, causal flash-style attention (unnormalized softmax: e = exp(s),
    y = (e @ v) * (1 / (e @ 1)) -- safe here because scores are O(1)),
  - partial output projection out_partial = y_g @ wproj[:, cols_g].T.

The host sums the 4 group partials per batch element.

All matmuls run in bf16 with fp32 PSUM accumulation.  Activations are cast
host-side; x / weights are pre-transposed host-side so the contraction dim
lands on SBUF partitions without any on-device fp32 transposes.
"""

import numpy as np
import ml_dtypes

BF16 = ml_dtypes.bfloat16

T = 2048
C = 2048
HD = 128
N_KV = 4
N_REP = 4
O_G = N_REP * HD  # 512 q-dims per group
TC = 512  # t-chunk (psum bank width in fp32)
N_TC = T // TC  # 4
N_KT = C // 128  # 16 contraction tiles
SCALE = float(1.0 / np.sqrt(HD))

_compiled = None


def _build():
    import concourse.bacc as bacc
    import concourse.mybir as mybir
    import concourse.tile as tile
    from concourse.masks import make_identity

    f32 = mybir.dt.float32
    bf16 = mybir.dt.bfloat16

    nc = bacc.Bacc("TRN2", target_bir_lowering=False, debug=False)

    xT = nc.dram_tensor("xT", [C, T], bf16, kind="ExternalInput").ap()
    wqT = nc.dram_tensor("wqT", [C, O_G], bf16, kind="ExternalInput").ap()
    wkT = nc.dram_tensor("wkT", [C, HD], bf16, kind="ExternalInput").ap()
    wvT = nc.dram_tensor("wvT", [C, HD], bf16, kind="ExternalInput").ap()
    wpT = nc.dram_tensor("wpT", [O_G, C], bf16, kind="ExternalInput").ap()
    bq = nc.dram_tensor("bq", [1, O_G], bf16, kind="ExternalInput").ap()
    bk = nc.dram_tensor("bk", [1, HD], bf16, kind="ExternalInput").ap()
    bv = nc.dram_tensor("bv", [1, HD], bf16, kind="ExternalInput").ap()
    ropeA = nc.dram_tensor("ropeA", [HD, T], f32, kind="ExternalInput").ap()
    ropeB = nc.dram_tensor("ropeB", [HD, T], f32, kind="ExternalInput").ap()
    masks = nc.dram_tensor("masks", [128, 4, TC], bf16, kind="ExternalInput").ap()
    out = nc.dram_tensor("out", [T, C], f32, kind="ExternalOutput").ap()

    Exp = mybir.ActivationFunctionType.Exp
    Copy = mybir.ActivationFunctionType.Copy

    with tile.TileContext(nc) as tc:
        import contextlib

        with contextlib.ExitStack() as ctx:
            persist = ctx.enter_context(tc.tile_pool(name="persist", bufs=1))

            # ---- persistent SBUF tensors ----
            wpT_sb = persist.tile([128, N_REP, C], bf16)
            qT_sb = persist.tile([128, N_REP, T], bf16)
            kT_sb = persist.tile([128, T], bf16)
            v_sb = persist.tile([128, N_KT, HD], bf16)
            yT_sb = persist.tile([128, N_REP, T], bf16)
            masks_sb = persist.tile([128, 4, TC], bf16)
            ones512 = persist.tile([1, TC], bf16)
            onescol = persist.tile([128, 1], bf16)
            onesrow = persist.tile([1, 128], bf16)
            ident = persist.tile([128, 128], bf16)

            # ---- phase B (projections): scoped pools ----
            bctx = contextlib.ExitStack()
            bpool = bctx.enter_context(tc.tile_pool(name="phase_b", bufs=1))
            tmp_pool = bctx.enter_context(tc.tile_pool(name="rope_tmp", bufs=2))
            xT_sb = bpool.tile([128, N_KT, T], bf16)
            wqT_sb = bpool.tile([128, N_KT, O_G], bf16)
            wkT_sb = bpool.tile([128, N_KT, HD], bf16)
            wvT_sb = bpool.tile([128, N_KT, HD], bf16)
            ropeA_sb = bpool.tile([128, T], f32)
            ropeB_sb = bpool.tile([128, T], f32)
            bq_sb = bpool.tile([1, O_G], bf16)
            bk_sb = bpool.tile([1, HD], bf16)
            bv_sb = bpool.tile([1, HD], bf16)
            vT_sb = bpool.tile([128, T], bf16)

            nc.vector.memset(ones512[:], 1.0)
            nc.vector.memset(onescol[:], 1.0)
            nc.vector.memset(onesrow[:], 1.0)
            make_identity(nc, ident[:])

            nc.sync.dma_start(masks_sb[:], masks[:])
            nc.sync.dma_start(wpT_sb[:], wpT.rearrange("(h p) m -> p h m", p=128))
            nc.sync.dma_start(wqT_sb[:], wqT.rearrange("(kt p) o -> p kt o", p=128))
            nc.sync.dma_start(wkT_sb[:], wkT.rearrange("(kt p) o -> p kt o", p=128))
            nc.sync.dma_start(wvT_sb[:], wvT.rearrange("(kt p) o -> p kt o", p=128))
            nc.sync.dma_start(bq_sb[:], bq[:])
            nc.sync.dma_start(bk_sb[:], bk[:])
            nc.sync.dma_start(bv_sb[:], bv[:])
            nc.sync.dma_start(ropeA_sb[:], ropeA[:])
            nc.sync.dma_start(ropeB_sb[:], ropeB[:])
            # x: one DMA per contraction tile so matmuls can start early
            for kt in range(N_KT):
                nc.sync.dma_start(
                    xT_sb[:, kt, :],
                    xT.rearrange("(kt p) t -> p kt t", p=128)[:, kt, :],
                )

            def rope_epilogue(ps, dst, t0):
                """dst (bf16 SBUF) = rope(ps) using A/B tables; ps is fp32 psum
                [128, TC] holding a projected qT/kT chunk at positions t0:t0+TC."""
                A = ropeA_sb[:, t0 : t0 + TC]
                Bm = ropeB_sb[:, t0 : t0 + TC]
                tmp = tmp_pool.tile([128, TC], f32, tag="rope_tmp")
                tmp2 = tmp_pool.tile([128, TC], f32, tag="rope_tmp2")
                nc.vector.tensor_mul(tmp[0:64, :], ps[64:128, :], Bm[0:64, :])
                nc.vector.tensor_mul(tmp[64:128, :], ps[0:64, :], Bm[64:128, :])
                nc.vector.tensor_mul(tmp2[:], ps[:], A)
                nc.vector.tensor_add(dst, tmp2[:], tmp[:])

            # ---- phase B1: q projection (transposed) + rope ----
            with tc.tile_pool(name="qpsum", bufs=8, space="PSUM") as qpsum:
                for tci in range(N_TC):
                    t0 = tci * TC
                    for ot in range(N_REP):
                        ps = qpsum.tile([128, TC], f32)
                        for kt in range(N_KT):
                            nc.tensor.matmul(
                                ps[:],
                                lhsT=wqT_sb[:, kt, ot * 128 : (ot + 1) * 128],
                                rhs=xT_sb[:, kt, t0 : t0 + TC],
                                start=(kt == 0),
                                stop=False,
                            )
                        nc.tensor.matmul(
                            ps[:],
                            lhsT=bq_sb[:, ot * 128 : (ot + 1) * 128],
                            rhs=ones512[:],
                            start=False,
                            stop=True,
                        )
                        rope_epilogue(ps, qT_sb[:, ot, t0 : t0 + TC], t0)

            # ---- phase B2: k, v projections ----
            with (
                tc.tile_pool(name="kpsum", bufs=2, space="PSUM") as kpsum,
                tc.tile_pool(name="vpsum", bufs=2, space="PSUM") as vpsum,
                tc.tile_pool(name="tpsum", bufs=2, space="PSUM") as tpsum,
            ):
                for tci in range(N_TC):
                    t0 = tci * TC
                    psk = kpsum.tile([128, TC], f32)
                    for kt in range(N_KT):
                        nc.tensor.matmul(
                            psk[:],
                            lhsT=wkT_sb[:, kt, :],
                            rhs=xT_sb[:, kt, t0 : t0 + TC],
                            start=(kt == 0),
                            stop=False,
                        )
                    nc.tensor.matmul(
                        psk[:], lhsT=bk_sb[:], rhs=ones512[:], start=False, stop=True
                    )
                    rope_epilogue(psk, kT_sb[:, t0 : t0 + TC], t0)

                    psv = vpsum.tile([128, TC], f32)
                    for kt in range(N_KT):
                        nc.tensor.matmul(
                            psv[:],
                            lhsT=wvT_sb[:, kt, :],
                            rhs=xT_sb[:, kt, t0 : t0 + TC],
                            start=(kt == 0),
                            stop=False,
                        )
                    nc.tensor.matmul(
                        psv[:], lhsT=bv_sb[:], rhs=ones512[:], start=False, stop=True
                    )
                    nc.scalar.copy(vT_sb[:, t0 : t0 + TC], psv[:])

                # transpose vT -> v (natural [t, d] tiles) via PE
                for jt in range(N_KT):
                    pst = tpsum.tile([128, 128], bf16)
                    nc.tensor.transpose(
                        pst[:], vT_sb[:, jt * 128 : (jt + 1) * 128], ident[:]
                    )
                    nc.scalar.copy(v_sb[:, jt, :], pst[:])

            bctx.close()

            # ---- phases C (attention) + D (output projection) ----
            small = ctx.enter_context(tc.tile_pool(name="small", bufs=2))
            stage_pool = ctx.enter_context(tc.tile_pool(name="stage", bufs=3))
            epool = ctx.enter_context(tc.tile_pool(name="e", bufs=4))
            with (
                tc.tile_pool(name="spsum", bufs=3, space="PSUM") as spsum,
                tc.tile_pool(name="ypsum", bufs=2, space="PSUM") as ypsum,
                tc.tile_pool(name="dpsum", bufs=1, space="PSUM") as dpsum,
                tc.tile_pool(name="opsum", bufs=2, space="PSUM") as opsum,
            ):
                Ln = mybir.ActivationFunctionType.Ln

                def emit_norm(p):
                    """Normalize head p: yT[h] = ps_y * broadcast(exp(-ln(den)))."""
                    lg = small.tile([1, TC], f32, tag="lg")
                    nc.scalar.activation(lg[:], p["ps_den"][:], Ln)
                    rden_bf = small.tile([1, TC], bf16, tag="rden_bf")
                    nc.scalar.activation(rden_bf[:], lg[:], Exp, scale=-1.0)
                    ps_rb = spsum.tile([128, TC], f32, tag="s")
                    nc.tensor.matmul(
                        ps_rb[:], lhsT=onesrow[:], rhs=rden_bf[:], start=True, stop=True
                    )
                    rb_sb = stage_pool.tile([128, TC], f32, tag="rb_stage")
                    nc.vector.tensor_copy(rb_sb[:], ps_rb[:])
                    nc.vector.tensor_mul(
                        yT_sb[:, p["h"], p["i0"] : p["i0"] + TC], p["ps_y"][:], rb_sb[:]
                    )

                def s_mm(h, i0, jt):
                    ps_s = spsum.tile([128, TC], f32, tag="s")
                    nc.tensor.matmul(
                        ps_s[:],
                        lhsT=kT_sb[:, jt * 128 : (jt + 1) * 128],
                        rhs=qT_sb[:, h, i0 : i0 + TC],
                        start=True,
                        stop=True,
                    )
                    return ps_s

                pending = None
                for ci in range(N_TC):
                    i0 = ci * TC
                    njt = 4 * (ci + 1)
                    for h in range(N_REP):
                        ps_y = ypsum.tile([128, TC], f32)
                        ps_den = dpsum.tile([1, TC], f32)
                        # depth-2 software pipeline on the score matmuls
                        s_tiles = {0: s_mm(h, i0, 0)}
                        if njt > 1:
                            s_tiles[1] = s_mm(h, i0, 1)
                        if pending is not None:
                            emit_norm(pending)
                            pending = None
                        for jt in range(njt):
                            ps_s = s_tiles.pop(jt)
                            e = epool.tile([128, TC], bf16)
                            nc.scalar.activation(e[:], ps_s[:], Exp, scale=SCALE)
                            r = jt - 4 * ci
                            if r >= 0:
                                nc.vector.tensor_mul(e[:], e[:], masks_sb[:, r, :])
                            nc.tensor.matmul(
                                ps_y[:],
                                lhsT=v_sb[:, jt, :],
                                rhs=e[:],
                                start=(jt == 0),
                                stop=(jt == njt - 1),
                            )
                            nc.tensor.matmul(
                                ps_den[:],
                                lhsT=onescol[:],
                                rhs=e[:],
                                start=(jt == 0),
                                stop=(jt == njt - 1),
                            )
                            if jt + 2 < njt:
                                s_tiles[jt + 2] = s_mm(h, i0, jt + 2)
                        pending = {"h": h, "i0": i0, "ps_y": ps_y, "ps_den": ps_den}

                    # finish the last head before the output projection needs it
                    emit_norm(pending)
                    pending = None

                    # phase D for the 4 t-subtiles of this chunk
                    for ts_ in range(4):
                        t_idx = ci * 4 + ts_
                        for mc in range(N_TC):
                            ps_o = opsum.tile([128, TC], f32)
                            for h in range(N_REP):
                                nc.tensor.matmul(
                                    ps_o[:],
                                    lhsT=yT_sb[:, h, t_idx * 128 : (t_idx + 1) * 128],
                                    rhs=wpT_sb[:, h, mc * TC : (mc + 1) * TC],
                                    start=(h == 0),
                                    stop=(h == N_REP - 1),
                                )
                            o_sb = stage_pool.tile([128, TC], f32, tag="o_stage")
                            # split psum->sbuf copies across ACT and DVE
                            if mc % 2 == 0:
                                nc.scalar.copy(o_sb[:], ps_o[:])
                            else:
                                nc.vector.tensor_copy(o_sb[:], ps_o[:])
                            nc.sync.dma_start(
                                out[
                                    t_idx * 128 : (t_idx + 1) * 128,
                                    mc * TC : (mc + 1) * TC,
                                ],
                                o_sb[:],
                            )

    nc.compile()
    return nc


def _get_compiled():
    global _compiled
    if _compiled is None:
        _compiled = _build()
    return _compiled


def kernel(x, cos, sin, wq, bq, wk, bk, wv, bv, wproj):
    from concourse.bass_utils import run_bass_kernel_spmd

    nc = _get_compiled()

    x = np.asarray(x, np.float32)
    cosT = np.asarray(cos, np.float32)[0, :, 0, :].T  # (64, T)
    sinT = np.asarray(sin, np.float32)[0, :, 0, :].T
    ropeA = np.ascontiguousarray(np.concatenate([cosT, cosT], 0))  # (128, T)
    ropeB = np.ascontiguousarray(np.concatenate([-sinT, sinT], 0))

    jj = np.arange(128, dtype=np.int64)[:, None, None]
    rr = np.arange(4, dtype=np.int64)[None, :, None]
    ii = np.arange(TC, dtype=np.int64)[None, None, :]
    masks = ((128 * rr + jj) <= ii).astype(BF16)  # (128, 4, 512)

    xT = [np.ascontiguousarray(x[b].T).astype(BF16) for b in range(2)]

    in_maps = []
    for c in range(8):
        b, g = divmod(c, 4)
        in_maps.append(
            {
                "xT": xT[b],
                "wqT": np.ascontiguousarray(
                    wq[g * O_G : (g + 1) * O_G].T
                ).astype(BF16),
                "wkT": np.ascontiguousarray(
                    wk[g * HD : (g + 1) * HD].T
                ).astype(BF16),
                "wvT": np.ascontiguousarray(
                    wv[g * HD : (g + 1) * HD].T
                ).astype(BF16),
                "wpT": np.ascontiguousarray(
                    wproj[:, g * O_G : (g + 1) * O_G].T
                ).astype(BF16),
                "bq": bq[None, g * O_G : (g + 1) * O_G].astype(BF16),
                "bk": bk[None, g * HD : (g + 1) * HD].astype(BF16),
                "bv": bv[None, g * HD : (g + 1) * HD].astype(BF16),
                "ropeA": ropeA,
                "ropeB": ropeB,
                "masks": masks,
            }
        )

    res = run_bass_kernel_spmd(nc, in_maps, core_ids=list(range(8)))
    parts = [res.results[c]["out"] for c in range(8)]
    out = np.stack(
        [
            parts[0] + parts[1] + parts[2] + parts[3],
            parts[4] + parts[5] + parts[6] + parts[7],
        ]
    ).astype(np.float32)
    return out


# revision 6
# speedup vs baseline: 1.1190x; 1.0427x over previous
"""Causal self-attention (GQA, rope) on 8 Trainium2 NeuronCores.

Sharding: tensor-parallel over the 4 kv-head groups x data-parallel over the
batch of 2.  Core c handles batch b = c // 4 and kv-group g = c % 4:

  - q/k/v projections for the group's 4 q-heads + 1 kv-head,
  - rope, causal flash-style attention (unnormalized softmax: e = exp(s),
    y = (e @ v) * (1 / (e @ 1)) -- safe here because scores are O(1)),
  - partial output projection out_partial = y_g @ wproj[:, cols_g].T.

The host sums the 4 group partials per batch element.

All matmuls run in bf16 with fp32 PSUM accumulation.  Activations are cast
host-side; x / weights are pre-transposed host-side so the contraction dim
lands on SBUF partitions without any on-device fp32 transposes.
"""

import numpy as np
import ml_dtypes

BF16 = ml_dtypes.bfloat16

T = 2048
C = 2048
HD = 128
N_KV = 4
N_REP = 4
O_G = N_REP * HD  # 512 q-dims per group
TC = 512  # t-chunk (psum bank width in fp32)
N_TC = T // TC  # 4
N_KT = C // 128  # 16 contraction tiles
SCALE = float(1.0 / np.sqrt(HD))

_compiled = None


def _build():
    import concourse.bacc as bacc
    import concourse.mybir as mybir
    import concourse.tile as tile
    from concourse.masks import make_identity

    f32 = mybir.dt.float32
    bf16 = mybir.dt.bfloat16

    nc = bacc.Bacc("TRN2", target_bir_lowering=False, debug=False)

    xT = nc.dram_tensor("xT", [C, T], bf16, kind="ExternalInput").ap()
    wqT = nc.dram_tensor("wqT", [C, O_G], bf16, kind="ExternalInput").ap()
    wkT = nc.dram_tensor("wkT", [C, HD], bf16, kind="ExternalInput").ap()
    wvT = nc.dram_tensor("wvT", [C, HD], bf16, kind="ExternalInput").ap()
    wpT = nc.dram_tensor("wpT", [O_G, C], bf16, kind="ExternalInput").ap()
    bq = nc.dram_tensor("bq", [1, O_G], bf16, kind="ExternalInput").ap()
    bk = nc.dram_tensor("bk", [1, HD], bf16, kind="ExternalInput").ap()
    bv = nc.dram_tensor("bv", [1, HD], bf16, kind="ExternalInput").ap()
    ropeA = nc.dram_tensor("ropeA", [HD, T], f32, kind="ExternalInput").ap()
    ropeB = nc.dram_tensor("ropeB", [HD, T], f32, kind="ExternalInput").ap()
    masks = nc.dram_tensor("masks", [128, 4, TC], bf16, kind="ExternalInput").ap()
    out = nc.dram_tensor("out", [T, C], f32, kind="ExternalOutput").ap()

    Exp = mybir.ActivationFunctionType.Exp
    Copy = mybir.ActivationFunctionType.Copy

    with tile.TileContext(nc) as tc:
        import contextlib

        with contextlib.ExitStack() as ctx:
            persist = ctx.enter_context(tc.tile_pool(name="persist", bufs=1))

            # ---- persistent SBUF tensors ----
            wpT_sb = persist.tile([128, N_REP, C], bf16)
            qT_sb = persist.tile([128, N_REP, T], bf16)
            kT_sb = persist.tile([128, T], bf16)
            v_sb = persist.tile([128, N_KT, HD], bf16)
            yT_sb = persist.tile([128, N_REP, T], bf16)
            masks_sb = persist.tile([128, 4, TC], bf16)
            ones512 = persist.tile([1, TC], bf16)
            onescol = persist.tile([128, 1], bf16)
            onesrow = persist.tile([1, 128], bf16)
            ident = persist.tile([128, 128], bf16)

            # ---- phase B (projections): scoped pools ----
            bctx = contextlib.ExitStack()
            bpool = bctx.enter_context(tc.tile_pool(name="phase_b", bufs=1))
            tmp_pool = bctx.enter_context(tc.tile_pool(name="rope_tmp", bufs=2))
            xT_sb = bpool.tile([128, N_KT, T], bf16)
            wqT_sb = bpool.tile([128, N_KT, O_G], bf16)
            wkT_sb = bpool.tile([128, N_KT, HD], bf16)
            wvT_sb = bpool.tile([128, N_KT, HD], bf16)
            ropeA_sb = bpool.tile([128, T], f32)
            ropeB_sb = bpool.tile([128, T], f32)
            bq_sb = bpool.tile([1, O_G], bf16)
            bk_sb = bpool.tile([1, HD], bf16)
            bv_sb = bpool.tile([1, HD], bf16)
            vT_sb = bpool.tile([128, T], bf16)

            nc.vector.memset(ones512[:], 1.0)
            nc.vector.memset(onescol[:], 1.0)
            nc.vector.memset(onesrow[:], 1.0)
            make_identity(nc, ident[:])

            nc.sync.dma_start(bq_sb[:], bq[:])
            nc.sync.dma_start(bk_sb[:], bk[:])
            nc.sync.dma_start(bv_sb[:], bv[:])
            nc.sync.dma_start(ropeA_sb[:], ropeA[:])
            nc.sync.dma_start(ropeB_sb[:], ropeB[:])
            # per-contraction-tile loads, in consumption order, so the first
            # projection matmuls start after ~640KB instead of ~13MB
            xT_r = xT.rearrange("(kt p) t -> p kt t", p=128)
            wqT_r = wqT.rearrange("(kt p) o -> p kt o", p=128)
            wkT_r = wkT.rearrange("(kt p) o -> p kt o", p=128)
            wvT_r = wvT.rearrange("(kt p) o -> p kt o", p=128)
            for kt in range(N_KT):
                nc.sync.dma_start(wqT_sb[:, kt, :], wqT_r[:, kt, :])
                nc.sync.dma_start(xT_sb[:, kt, :], xT_r[:, kt, :])
                nc.sync.dma_start(wkT_sb[:, kt, :], wkT_r[:, kt, :])
                nc.sync.dma_start(wvT_sb[:, kt, :], wvT_r[:, kt, :])
            nc.sync.dma_start(masks_sb[:], masks[:])
            nc.sync.dma_start(wpT_sb[:], wpT.rearrange("(h p) m -> p h m", p=128))

            def rope_epilogue(ps, dst, t0):
                """dst (bf16 SBUF) = rope(ps) using A/B tables; ps is fp32 psum
                [128, TC] holding a projected qT/kT chunk at positions t0:t0+TC."""
                A = ropeA_sb[:, t0 : t0 + TC]
                Bm = ropeB_sb[:, t0 : t0 + TC]
                tmp = tmp_pool.tile([128, TC], f32, tag="rope_tmp")
                tmp2 = tmp_pool.tile([128, TC], f32, tag="rope_tmp2")
                nc.vector.tensor_mul(tmp[0:64, :], ps[64:128, :], Bm[0:64, :])
                nc.vector.tensor_mul(tmp[64:128, :], ps[0:64, :], Bm[64:128, :])
                nc.vector.tensor_mul(tmp2[:], ps[:], A)
                nc.vector.tensor_add(dst, tmp2[:], tmp[:])

            # ---- phase B1: q projection (transposed) + rope ----
            with tc.tile_pool(name="qpsum", bufs=8, space="PSUM") as qpsum:
                for tci in range(N_TC):
                    t0 = tci * TC
                    for ot in range(N_REP):
                        ps = qpsum.tile([128, TC], f32)
                        for kt in range(N_KT):
                            nc.tensor.matmul(
                                ps[:],
                                lhsT=wqT_sb[:, kt, ot * 128 : (ot + 1) * 128],
                                rhs=xT_sb[:, kt, t0 : t0 + TC],
                                start=(kt == 0),
                                stop=False,
                            )
                        nc.tensor.matmul(
                            ps[:],
                            lhsT=bq_sb[:, ot * 128 : (ot + 1) * 128],
                            rhs=ones512[:],
                            start=False,
                            stop=True,
                        )
                        rope_epilogue(ps, qT_sb[:, ot, t0 : t0 + TC], t0)

            # ---- phase B2: k, v projections ----
            with (
                tc.tile_pool(name="kpsum", bufs=2, space="PSUM") as kpsum,
                tc.tile_pool(name="vpsum", bufs=2, space="PSUM") as vpsum,
                tc.tile_pool(name="tpsum", bufs=2, space="PSUM") as tpsum,
            ):
                for tci in range(N_TC):
                    t0 = tci * TC
                    psk = kpsum.tile([128, TC], f32)
                    for kt in range(N_KT):
                        nc.tensor.matmul(
                            psk[:],
                            lhsT=wkT_sb[:, kt, :],
                            rhs=xT_sb[:, kt, t0 : t0 + TC],
                            start=(kt == 0),
                            stop=False,
                        )
                    nc.tensor.matmul(
                        psk[:], lhsT=bk_sb[:], rhs=ones512[:], start=False, stop=True
                    )
                    rope_epilogue(psk, kT_sb[:, t0 : t0 + TC], t0)

                    psv = vpsum.tile([128, TC], f32)
                    for kt in range(N_KT):
                        nc.tensor.matmul(
                            psv[:],
                            lhsT=wvT_sb[:, kt, :],
                            rhs=xT_sb[:, kt, t0 : t0 + TC],
                            start=(kt == 0),
                            stop=False,
                        )
                    nc.tensor.matmul(
                        psv[:], lhsT=bv_sb[:], rhs=ones512[:], start=False, stop=True
                    )
                    nc.scalar.copy(vT_sb[:, t0 : t0 + TC], psv[:])

                # transpose vT -> v (natural [t, d] tiles) via PE
                for jt in range(N_KT):
                    pst = tpsum.tile([128, 128], bf16)
                    nc.tensor.transpose(
                        pst[:], vT_sb[:, jt * 128 : (jt + 1) * 128], ident[:]
                    )
                    nc.scalar.copy(v_sb[:, jt, :], pst[:])

            bctx.close()

            # ---- phase C (attention) ----
            small = ctx.enter_context(tc.tile_pool(name="small", bufs=2))
            stage_pool = ctx.enter_context(tc.tile_pool(name="stage", bufs=4))
            epool = ctx.enter_context(tc.tile_pool(name="e", bufs=4))
            Ln = mybir.ActivationFunctionType.Ln
            with (
                tc.tile_pool(name="spsum", bufs=4, space="PSUM") as spsum,
                tc.tile_pool(name="ypsum", bufs=2, space="PSUM") as ypsum,
                tc.tile_pool(name="dpsum", bufs=2, space="PSUM") as dpsum,
            ):

                def emit_norm(p):
                    """Normalize head p: yT[h] = ps_y * broadcast(exp(-ln(den)))."""
                    lg = small.tile([1, TC], f32, tag="lg")
                    nc.scalar.activation(lg[:], p["ps_den"][:], Ln)
                    rden_bf = small.tile([1, TC], bf16, tag="rden_bf")
                    nc.scalar.activation(rden_bf[:], lg[:], Exp, scale=-1.0)
                    ps_rb = spsum.tile([128, TC], f32, tag="s")
                    nc.tensor.matmul(
                        ps_rb[:], lhsT=onesrow[:], rhs=rden_bf[:], start=True, stop=True
                    )
                    rb_sb = stage_pool.tile([128, TC], f32, tag="rb_stage")
                    nc.vector.tensor_copy(rb_sb[:], ps_rb[:])
                    nc.vector.tensor_mul(
                        yT_sb[:, p["h"], p["i0"] : p["i0"] + TC], p["ps_y"][:], rb_sb[:]
                    )

                def s_mm(h, i0, jt):
                    ps_s = spsum.tile([128, TC], f32, tag="s")
                    nc.tensor.matmul(
                        ps_s[:],
                        lhsT=kT_sb[:, jt * 128 : (jt + 1) * 128],
                        rhs=qT_sb[:, h, i0 : i0 + TC],
                        start=True,
                        stop=True,
                    )
                    return ps_s

                DEPTH = 3
                pending = None
                for ci in range(N_TC):
                    i0 = ci * TC
                    njt = 4 * (ci + 1)
                    for h in range(N_REP):
                        ps_y = ypsum.tile([128, TC], f32)
                        ps_den = dpsum.tile([1, TC], f32)
                        s_tiles = {}
                        for jt in range(min(DEPTH, njt)):
                            s_tiles[jt] = s_mm(h, i0, jt)
                        if pending is not None:
                            emit_norm(pending)
                            pending = None
                        for jt in range(njt):
                            ps_s = s_tiles.pop(jt)
                            e = epool.tile([128, TC], bf16)
                            nc.scalar.activation(e[:], ps_s[:], Exp, scale=SCALE)
                            r = jt - 4 * ci
                            if r >= 0:
                                nc.vector.tensor_mul(e[:], e[:], masks_sb[:, r, :])
                            nc.tensor.matmul(
                                ps_y[:],
                                lhsT=v_sb[:, jt, :],
                                rhs=e[:],
                                start=(jt == 0),
                                stop=(jt == njt - 1),
                            )
                            nc.tensor.matmul(
                                ps_den[:],
                                lhsT=onescol[:],
                                rhs=e[:],
                                start=(jt == 0),
                                stop=(jt == njt - 1),
                            )
                            if jt + DEPTH < njt:
                                s_tiles[jt + DEPTH] = s_mm(h, i0, jt + DEPTH)
                        pending = {"h": h, "i0": i0, "ps_y": ps_y, "ps_den": ps_den}
                emit_norm(pending)
                pending = None

            # ---- phase D (output projection), PE-bound on its own ----
            with tc.tile_pool(name="opsum", bufs=4, space="PSUM") as opsum:
                for t_idx in range(T // 128):
                    for mc in range(N_TC):
                        ps_o = opsum.tile([128, TC], f32)
                        for h in range(N_REP):
                            nc.tensor.matmul(
                                ps_o[:],
                                lhsT=yT_sb[:, h, t_idx * 128 : (t_idx + 1) * 128],
                                rhs=wpT_sb[:, h, mc * TC : (mc + 1) * TC],
                                start=(h == 0),
                                stop=(h == N_REP - 1),
                            )
                        o_sb = stage_pool.tile([128, TC], f32, tag="o_stage")
                        # split psum->sbuf copies across ACT and DVE
                        if mc % 2 == 0:
                            nc.scalar.copy(o_sb[:], ps_o[:])
                        else:
                            nc.vector.tensor_copy(o_sb[:], ps_o[:])
                        nc.sync.dma_start(
                            out[
                                t_idx * 128 : (t_idx + 1) * 128,
                                mc * TC : (mc + 1) * TC,
                            ],
                            o_sb[:],
                        )

    nc.compile()
    return nc


def _get_compiled():
    global _compiled
    if _compiled is None:
        _compiled = _build()
    return _compiled


def kernel(x, cos, sin, wq, bq, wk, bk, wv, bv, wproj):
    from concourse.bass_utils import run_bass_kernel_spmd

    nc = _get_compiled()

    x = np.asarray(x, np.float32)
    cosT = np.asarray(cos, np.float32)[0, :, 0, :].T  # (64, T)
    sinT = np.asarray(sin, np.float32)[0, :, 0, :].T
    ropeA = np.ascontiguousarray(np.concatenate([cosT, cosT], 0))  # (128, T)
    ropeB = np.ascontiguousarray(np.concatenate([-sinT, sinT], 0))

    jj = np.arange(128, dtype=np.int64)[:, None, None]
    rr = np.arange(4, dtype=np.int64)[None, :, None]
    ii = np.arange(TC, dtype=np.int64)[None, None, :]
    masks = ((128 * rr + jj) <= ii).astype(BF16)  # (128, 4, 512)

    xT = [np.ascontiguousarray(x[b].T).astype(BF16) for b in range(2)]

    in_maps = []
    for c in range(8):
        b, g = divmod(c, 4)
        in_maps.append(
            {
                "xT": xT[b],
                "wqT": np.ascontiguousarray(
                    wq[g * O_G : (g + 1) * O_G].T
                ).astype(BF16),
                "wkT": np.ascontiguousarray(
                    wk[g * HD : (g + 1) * HD].T
                ).astype(BF16),
                "wvT": np.ascontiguousarray(
                    wv[g * HD : (g + 1) * HD].T
                ).astype(BF16),
                "wpT": np.ascontiguousarray(
                    wproj[:, g * O_G : (g + 1) * O_G].T
                ).astype(BF16),
                "bq": bq[None, g * O_G : (g + 1) * O_G].astype(BF16),
                "bk": bk[None, g * HD : (g + 1) * HD].astype(BF16),
                "bv": bv[None, g * HD : (g + 1) * HD].astype(BF16),
                "ropeA": ropeA,
                "ropeB": ropeB,
                "masks": masks,
            }
        )

    res = run_bass_kernel_spmd(nc, in_maps, core_ids=list(range(8)))
    parts = [res.results[c]["out"] for c in range(8)]
    out = np.stack(
        [
            parts[0] + parts[1] + parts[2] + parts[3],
            parts[4] + parts[5] + parts[6] + parts[7],
        ]
    ).astype(np.float32)
    return out


# revision 8
# speedup vs baseline: 1.1877x; 1.0614x over previous
"""Causal self-attention (GQA, rope) on 8 Trainium2 NeuronCores.

Sharding: tensor-parallel over the 4 kv-head groups x data-parallel over the
batch of 2.  Core c handles batch b = c // 4 and kv-group g = c % 4:

  - q/k/v projections for the group's 4 q-heads + 1 kv-head,
  - rope, causal flash-style attention (unnormalized softmax: e = exp(s),
    y = (e @ v) * (1 / (e @ 1)) -- safe here because scores are O(1)),
  - partial output projection out_partial = y_g @ wproj[:, cols_g].T.

The host sums the 4 group partials per batch element.

All matmuls run in bf16 with fp32 PSUM accumulation.  Activations are cast
host-side; x / weights are pre-transposed host-side so the contraction dim
lands on SBUF partitions without any on-device fp32 transposes.
"""

import numpy as np
import ml_dtypes

BF16 = ml_dtypes.bfloat16

T = 2048
C = 2048
HD = 128
N_KV = 4
N_REP = 4
O_G = N_REP * HD  # 512 q-dims per group
TC = 512  # t-chunk (psum bank width in fp32)
N_TC = T // TC  # 4
N_KT = C // 128  # 16 contraction tiles
SCALE = float(1.0 / np.sqrt(HD))

_compiled = None


def _build():
    import concourse.bacc as bacc
    import concourse.mybir as mybir
    import concourse.tile as tile
    from concourse.masks import make_identity

    f32 = mybir.dt.float32
    bf16 = mybir.dt.bfloat16

    nc = bacc.Bacc("TRN2", target_bir_lowering=False, debug=False)

    xT = nc.dram_tensor("xT", [C, T], bf16, kind="ExternalInput").ap()
    wqT = nc.dram_tensor("wqT", [C, O_G], bf16, kind="ExternalInput").ap()
    wkT = nc.dram_tensor("wkT", [C, HD], bf16, kind="ExternalInput").ap()
    wvT = nc.dram_tensor("wvT", [C, HD], bf16, kind="ExternalInput").ap()
    wpT = nc.dram_tensor("wpT", [O_G, C], bf16, kind="ExternalInput").ap()
    bq = nc.dram_tensor("bq", [1, O_G], bf16, kind="ExternalInput").ap()
    bk = nc.dram_tensor("bk", [1, HD], bf16, kind="ExternalInput").ap()
    bv = nc.dram_tensor("bv", [1, HD], bf16, kind="ExternalInput").ap()
    ropeA = nc.dram_tensor("ropeA", [HD, T], f32, kind="ExternalInput").ap()
    ropeB = nc.dram_tensor("ropeB", [HD, T], f32, kind="ExternalInput").ap()
    masks = nc.dram_tensor("masks", [128, 4, TC], bf16, kind="ExternalInput").ap()
    out = nc.dram_tensor("out", [T, C], f32, kind="ExternalOutput").ap()

    Exp = mybir.ActivationFunctionType.Exp
    Copy = mybir.ActivationFunctionType.Copy

    with tile.TileContext(nc) as tc:
        import contextlib

        with contextlib.ExitStack() as ctx:
            persist = ctx.enter_context(tc.tile_pool(name="persist", bufs=1))

            # ---- persistent SBUF tensors ----
            wpT_sb = persist.tile([128, N_REP, C], bf16)
            qT_sb = persist.tile([128, N_REP, T], bf16)
            kT_sb = persist.tile([128, T], bf16)
            v_sb = persist.tile([128, N_KT, HD], bf16)
            yT_sb = persist.tile([128, N_REP, T], bf16)
            masks_sb = persist.tile([128, 4, TC], bf16)
            ones512 = persist.tile([1, TC], bf16)
            onescol = persist.tile([128, 1], bf16)
            onesrow = persist.tile([1, 128], bf16)
            ident = persist.tile([128, 128], bf16)

            # ---- phase B (projections): scoped pools ----
            bctx = contextlib.ExitStack()
            bpool = bctx.enter_context(tc.tile_pool(name="phase_b", bufs=1))
            tmp_pool = bctx.enter_context(tc.tile_pool(name="rope_tmp", bufs=2))
            xT_sb = bpool.tile([128, N_KT, T], bf16)
            wqT_sb = bpool.tile([128, N_KT, O_G], bf16)
            wkT_sb = bpool.tile([128, N_KT, HD], bf16)
            wvT_sb = bpool.tile([128, N_KT, HD], bf16)
            ropeA_sb = bpool.tile([128, T], f32)
            ropeB_sb = bpool.tile([128, T], f32)
            bq_sb = bpool.tile([1, O_G], bf16)
            bk_sb = bpool.tile([1, HD], bf16)
            bv_sb = bpool.tile([1, HD], bf16)
            vT_sb = bpool.tile([128, T], bf16)

            nc.vector.memset(ones512[:], 1.0)
            nc.vector.memset(onescol[:], 1.0)
            nc.vector.memset(onesrow[:], 1.0)
            make_identity(nc, ident[:])

            nc.sync.dma_start(bq_sb[:], bq[:])
            nc.sync.dma_start(bk_sb[:], bk[:])
            nc.sync.dma_start(bv_sb[:], bv[:])
            nc.sync.dma_start(ropeA_sb[:], ropeA[:])
            nc.sync.dma_start(ropeB_sb[:], ropeB[:])
            # per-contraction-tile loads, in consumption order, so the first
            # projection matmuls start after ~640KB instead of ~13MB
            xT_r = xT.rearrange("(kt p) t -> p kt t", p=128)
            wqT_r = wqT.rearrange("(kt p) o -> p kt o", p=128)
            wkT_r = wkT.rearrange("(kt p) o -> p kt o", p=128)
            wvT_r = wvT.rearrange("(kt p) o -> p kt o", p=128)
            for kt in range(N_KT):
                nc.sync.dma_start(wqT_sb[:, kt, :], wqT_r[:, kt, :])
                nc.sync.dma_start(xT_sb[:, kt, :], xT_r[:, kt, :])
                nc.sync.dma_start(wkT_sb[:, kt, :], wkT_r[:, kt, :])
                nc.sync.dma_start(wvT_sb[:, kt, :], wvT_r[:, kt, :])
            nc.sync.dma_start(masks_sb[:], masks[:])
            nc.sync.dma_start(wpT_sb[:], wpT.rearrange("(h p) m -> p h m", p=128))

            def rope_epilogue(ps, dst, t0):
                """dst (bf16 SBUF) = rope(ps) using A/B tables; ps is fp32 psum
                [128, TC] holding a projected qT/kT chunk at positions t0:t0+TC."""
                A = ropeA_sb[:, t0 : t0 + TC]
                Bm = ropeB_sb[:, t0 : t0 + TC]
                tmp = tmp_pool.tile([128, TC], f32, tag="rope_tmp")
                tmp2 = tmp_pool.tile([128, TC], f32, tag="rope_tmp2")
                nc.vector.tensor_mul(tmp[0:64, :], ps[64:128, :], Bm[0:64, :])
                nc.vector.tensor_mul(tmp[64:128, :], ps[0:64, :], Bm[64:128, :])
                nc.vector.tensor_mul(tmp2[:], ps[:], A)
                nc.vector.tensor_add(dst, tmp2[:], tmp[:])

            # ---- phase B1: q projection (transposed) + rope ----
            with tc.tile_pool(name="qpsum", bufs=8, space="PSUM") as qpsum:
                for tci in range(N_TC):
                    t0 = tci * TC
                    for ot in range(N_REP):
                        ps = qpsum.tile([128, TC], f32)
                        for kt in range(N_KT):
                            nc.tensor.matmul(
                                ps[:],
                                lhsT=wqT_sb[:, kt, ot * 128 : (ot + 1) * 128],
                                rhs=xT_sb[:, kt, t0 : t0 + TC],
                                start=(kt == 0),
                                stop=False,
                            )
                        nc.tensor.matmul(
                            ps[:],
                            lhsT=bq_sb[:, ot * 128 : (ot + 1) * 128],
                            rhs=ones512[:],
                            start=False,
                            stop=True,
                        )
                        rope_epilogue(ps, qT_sb[:, ot, t0 : t0 + TC], t0)

            # ---- phase B2: k, v projections ----
            with (
                tc.tile_pool(name="kpsum", bufs=2, space="PSUM") as kpsum,
                tc.tile_pool(name="vpsum", bufs=2, space="PSUM") as vpsum,
                tc.tile_pool(name="tpsum", bufs=2, space="PSUM") as tpsum,
            ):
                for tci in range(N_TC):
                    t0 = tci * TC
                    psk = kpsum.tile([128, TC], f32)
                    for kt in range(N_KT):
                        nc.tensor.matmul(
                            psk[:],
                            lhsT=wkT_sb[:, kt, :],
                            rhs=xT_sb[:, kt, t0 : t0 + TC],
                            start=(kt == 0),
                            stop=False,
                        )
                    nc.tensor.matmul(
                        psk[:], lhsT=bk_sb[:], rhs=ones512[:], start=False, stop=True
                    )
                    rope_epilogue(psk, kT_sb[:, t0 : t0 + TC], t0)

                    psv = vpsum.tile([128, TC], f32)
                    for kt in range(N_KT):
                        nc.tensor.matmul(
                            psv[:],
                            lhsT=wvT_sb[:, kt, :],
                            rhs=xT_sb[:, kt, t0 : t0 + TC],
                            start=(kt == 0),
                            stop=False,
                        )
                    nc.tensor.matmul(
                        psv[:], lhsT=bv_sb[:], rhs=ones512[:], start=False, stop=True
                    )
                    nc.scalar.copy(vT_sb[:, t0 : t0 + TC], psv[:])

                # transpose vT -> v (natural [t, d] tiles) via PE
                for jt in range(N_KT):
                    pst = tpsum.tile([128, 128], bf16)
                    nc.tensor.transpose(
                        pst[:], vT_sb[:, jt * 128 : (jt + 1) * 128], ident[:]
                    )
                    nc.scalar.copy(v_sb[:, jt, :], pst[:])

            bctx.close()

            # ---- phase C (attention) ----
            small = ctx.enter_context(tc.tile_pool(name="small", bufs=2))
            stage_pool = ctx.enter_context(tc.tile_pool(name="stage", bufs=4))
            epool = ctx.enter_context(tc.tile_pool(name="e", bufs=3))
            cpool = ctx.enter_context(tc.tile_pool(name="cphase", bufs=1))
            dci_pool = ctx.enter_context(tc.tile_pool(name="dci", bufs=2))
            Ln = mybir.ActivationFunctionType.Ln

            # unnormalized attention outputs, fp32
            yun_sb = cpool.tile([128, N_REP, T], f32)

            with (
                tc.tile_pool(name="spsum", bufs=2, space="PSUM") as spsum,
                tc.tile_pool(name="ypsum", bufs=2, space="PSUM") as ypsum,
                tc.tile_pool(name="dpsum", bufs=2, space="PSUM") as dpsum,
            ):

                def s_group(h, i0, g):
                    """Two score matmuls (key tiles 2g, 2g+1) into one 2-bank
                    psum tile, so exp can run 1024 wide."""
                    ps = spsum.tile([128, 2, TC], f32, tag="s")
                    for sub in range(2):
                        jt = 2 * g + sub
                        nc.tensor.matmul(
                            ps[:, sub, :],
                            lhsT=kT_sb[:, jt * 128 : (jt + 1) * 128],
                            rhs=qT_sb[:, h, i0 : i0 + TC],
                            start=True,
                            stop=True,
                        )
                    return ps

                def make_norm_thunks(ci, den_ci, i0):
                    rden_bf = dci_pool.tile([1, N_REP * TC], bf16, tag="rden")
                    lg = dci_pool.tile([1, N_REP * TC], f32, tag="lg")

                    def lnexp():
                        nc.scalar.activation(lg[:], den_ci[:], Ln)
                        nc.scalar.activation(rden_bf[:], lg[:], Exp, scale=-1.0)

                    def mk_h(h):
                        def norm_h():
                            ps_rb = spsum.tile([128, 2, TC], f32, tag="s")
                            nc.tensor.matmul(
                                ps_rb[:, 0, :],
                                lhsT=onesrow[:],
                                rhs=rden_bf[0:1, h * TC : (h + 1) * TC],
                                start=True,
                                stop=True,
                            )
                            rb_sb = stage_pool.tile([128, TC], f32, tag="rb_stage")
                            nc.vector.tensor_copy(rb_sb[:], ps_rb[:, 0, :])
                            nc.vector.tensor_mul(
                                yT_sb[:, h, i0 : i0 + TC],
                                yun_sb[:, h, i0 : i0 + TC],
                                rb_sb[:],
                            )

                        return norm_h

                    return [lnexp] + [mk_h(h) for h in range(N_REP)]

                norm_queue = []
                for ci in range(N_TC):
                    i0 = ci * TC
                    ng = 2 * (ci + 1)
                    den_ci = dci_pool.tile([1, N_REP * TC], f32, tag="den_ci")
                    for h in range(N_REP):
                        ps_y = ypsum.tile([128, TC], f32)
                        ps_den = dpsum.tile([1, TC], f32)
                        s_tiles = {0: s_group(h, i0, 0)}
                        if ng > 1:
                            s_tiles[1] = s_group(h, i0, 1)
                        # drain deferred normalization work from the previous chunk
                        for _ in range(2):
                            if norm_queue:
                                norm_queue.pop(0)()
                        for g in range(ng):
                            ps_s = s_tiles.pop(g)
                            e = epool.tile([128, 2, TC], bf16)
                            nc.scalar.activation(e[:], ps_s[:], Exp, scale=SCALE)
                            dg = g - 2 * ci
                            if dg >= 0:
                                nc.vector.tensor_mul(
                                    e[:], e[:], masks_sb[:, 2 * dg : 2 * dg + 2, :]
                                )
                            for sub in range(2):
                                jt = 2 * g + sub
                                nc.tensor.matmul(
                                    ps_y[:],
                                    lhsT=v_sb[:, jt, :],
                                    rhs=e[:, sub, :],
                                    start=(jt == 0),
                                    stop=(jt == 2 * ng - 1),
                                )
                                nc.tensor.matmul(
                                    ps_den[:],
                                    lhsT=onescol[:],
                                    rhs=e[:, sub, :],
                                    start=(jt == 0),
                                    stop=(jt == 2 * ng - 1),
                                )
                            if g + 2 < ng:
                                s_tiles[g + 2] = s_group(h, i0, g + 2)
                        # stage unnormalized outputs; free the psum banks fast
                        nc.vector.tensor_copy(
                            den_ci[0:1, h * TC : (h + 1) * TC], ps_den[:]
                        )
                        nc.vector.tensor_copy(yun_sb[:, h, i0 : i0 + TC], ps_y[:])
                    norm_queue.extend(make_norm_thunks(ci, den_ci, i0))
                while norm_queue:
                    norm_queue.pop(0)()

            # ---- phase D (output projection), PE-bound on its own ----
            with tc.tile_pool(name="opsum", bufs=4, space="PSUM") as opsum:
                for t_idx in range(T // 128):
                    for mc in range(N_TC):
                        ps_o = opsum.tile([128, TC], f32)
                        for h in range(N_REP):
                            nc.tensor.matmul(
                                ps_o[:],
                                lhsT=yT_sb[:, h, t_idx * 128 : (t_idx + 1) * 128],
                                rhs=wpT_sb[:, h, mc * TC : (mc + 1) * TC],
                                start=(h == 0),
                                stop=(h == N_REP - 1),
                            )
                        o_sb = stage_pool.tile([128, TC], f32, tag="o_stage")
                        # split psum->sbuf copies across ACT and DVE
                        if mc % 2 == 0:
                            nc.scalar.copy(o_sb[:], ps_o[:])
                        else:
                            nc.vector.tensor_copy(o_sb[:], ps_o[:])
                        nc.sync.dma_start(
                            out[
                                t_idx * 128 : (t_idx + 1) * 128,
                                mc * TC : (mc + 1) * TC,
                            ],
                            o_sb[:],
                        )

    nc.compile()
    return nc


def _get_compiled():
    global _compiled
    if _compiled is None:
        _compiled = _build()
    return _compiled


def kernel(x, cos, sin, wq, bq, wk, bk, wv, bv, wproj):
    from concourse.bass_utils import run_bass_kernel_spmd

    nc = _get_compiled()

    x = np.asarray(x, np.float32)
    cosT = np.asarray(cos, np.float32)[0, :, 0, :].T  # (64, T)
    sinT = np.asarray(sin, np.float32)[0, :, 0, :].T
    ropeA = np.ascontiguousarray(np.concatenate([cosT, cosT], 0))  # (128, T)
    ropeB = np.ascontiguousarray(np.concatenate([-sinT, sinT], 0))

    jj = np.arange(128, dtype=np.int64)[:, None, None]
    rr = np.arange(4, dtype=np.int64)[None, :, None]
    ii = np.arange(TC, dtype=np.int64)[None, None, :]
    masks = ((128 * rr + jj) <= ii).astype(BF16)  # (128, 4, 512)

    xT = [np.ascontiguousarray(x[b].T).astype(BF16) for b in range(2)]

    in_maps = []
    for c in range(8):
        b, g = divmod(c, 4)
        in_maps.append(
            {
                "xT": xT[b],
                "wqT": np.ascontiguousarray(
                    wq[g * O_G : (g + 1) * O_G].T
                ).astype(BF16),
                "wkT": np.ascontiguousarray(
                    wk[g * HD : (g + 1) * HD].T
                ).astype(BF16),
                "wvT": np.ascontiguousarray(
                    wv[g * HD : (g + 1) * HD].T
                ).astype(BF16),
                "wpT": np.ascontiguousarray(
                    wproj[:, g * O_G : (g + 1) * O_G].T
                ).astype(BF16),
                "bq": bq[None, g * O_G : (g + 1) * O_G].astype(BF16),
                "bk": bk[None, g * HD : (g + 1) * HD].astype(BF16),
                "bv": bv[None, g * HD : (g + 1) * HD].astype(BF16),
                "ropeA": ropeA,
                "ropeB": ropeB,
                "masks": masks,
            }
        )

    res = run_bass_kernel_spmd(nc, in_maps, core_ids=list(range(8)))
    parts = [res.results[c]["out"] for c in range(8)]
    out = np.stack(
        [
            parts[0] + parts[1] + parts[2] + parts[3],
            parts[4] + parts[5] + parts[6] + parts[7],
        ]
    ).astype(np.float32)
    return out


# revision 9
# speedup vs baseline: 1.2698x; 1.0691x over previous
"""Causal self-attention (GQA, rope) on 8 Trainium2 NeuronCores.

Sharding: tensor-parallel over the 4 kv-head groups x data-parallel over the
batch of 2.  Core c handles batch b = c // 4 and kv-group g = c % 4:

  - q/k/v projections for the group's 4 q-heads + 1 kv-head,
  - rope, causal flash-style attention (unnormalized softmax: e = exp(s),
    y = (e @ v) * (1 / (e @ 1)) -- safe here because scores are O(1)),
  - partial output projection out_partial = y_g @ wproj[:, cols_g].T.

The host sums the 4 group partials per batch element.

All matmuls run in bf16 with fp32 PSUM accumulation.  Activations are cast
host-side; x / weights are pre-transposed host-side so the contraction dim
lands on SBUF partitions without any on-device fp32 transposes.
"""

import numpy as np
import ml_dtypes

BF16 = ml_dtypes.bfloat16

T = 2048
C = 2048
HD = 128
N_KV = 4
N_REP = 4
O_G = N_REP * HD  # 512 q-dims per group
TC = 512  # t-chunk (psum bank width in fp32)
N_TC = T // TC  # 4
N_KT = C // 128  # 16 contraction tiles
SCALE = float(1.0 / np.sqrt(HD))

_compiled = None


def _build():
    import concourse.bacc as bacc
    import concourse.mybir as mybir
    import concourse.tile as tile
    from concourse.masks import make_identity

    f32 = mybir.dt.float32
    bf16 = mybir.dt.bfloat16

    nc = bacc.Bacc("TRN2", target_bir_lowering=False, debug=False)

    xT = nc.dram_tensor("xT", [C, T], bf16, kind="ExternalInput").ap()
    wqT = nc.dram_tensor("wqT", [C, O_G], bf16, kind="ExternalInput").ap()
    wkT = nc.dram_tensor("wkT", [C, HD], bf16, kind="ExternalInput").ap()
    wvT = nc.dram_tensor("wvT", [C, HD], bf16, kind="ExternalInput").ap()
    wpT = nc.dram_tensor("wpT", [O_G, C], bf16, kind="ExternalInput").ap()
    bq = nc.dram_tensor("bq", [1, O_G], bf16, kind="ExternalInput").ap()
    bk = nc.dram_tensor("bk", [1, HD], bf16, kind="ExternalInput").ap()
    bv = nc.dram_tensor("bv", [1, HD], bf16, kind="ExternalInput").ap()
    ropeA = nc.dram_tensor("ropeA", [HD, T], f32, kind="ExternalInput").ap()
    ropeB = nc.dram_tensor("ropeB", [HD, T], f32, kind="ExternalInput").ap()
    masks = nc.dram_tensor("masks", [128, 4, TC], bf16, kind="ExternalInput").ap()
    out = nc.dram_tensor("out", [T, C], f32, kind="ExternalOutput").ap()

    Exp = mybir.ActivationFunctionType.Exp
    Copy = mybir.ActivationFunctionType.Copy

    with tile.TileContext(nc) as tc:
        import contextlib

        with contextlib.ExitStack() as ctx:
            persist = ctx.enter_context(tc.tile_pool(name="persist", bufs=1))

            # ---- persistent SBUF tensors ----
            wpT_sb = persist.tile([128, N_REP, C], bf16)
            qT_sb = persist.tile([128, N_REP, T], bf16)
            kT_sb = persist.tile([128, T], bf16)
            v_sb = persist.tile([128, N_KT, HD], bf16)
            yT_sb = persist.tile([128, N_REP, T], bf16)
            masks_sb = persist.tile([128, 4, TC], bf16)
            ones512 = persist.tile([1, TC], bf16)
            onescol = persist.tile([128, 1], bf16)
            onesrow = persist.tile([1, 128], bf16)
            ident = persist.tile([128, 128], bf16)

            # ---- phase B (projections): scoped pools ----
            bctx = contextlib.ExitStack()
            bpool = bctx.enter_context(tc.tile_pool(name="phase_b", bufs=1))
            tmp_pool = bctx.enter_context(tc.tile_pool(name="rope_tmp", bufs=2))
            xT_sb = bpool.tile([128, N_KT, T], bf16)
            wqT_sb = bpool.tile([128, N_KT, O_G], bf16)
            wkT_sb = bpool.tile([128, N_KT, HD], bf16)
            wvT_sb = bpool.tile([128, N_KT, HD], bf16)
            ropeA_sb = bpool.tile([128, T], f32)
            ropeB_sb = bpool.tile([128, T], f32)
            bq_sb = bpool.tile([1, O_G], bf16)
            bk_sb = bpool.tile([1, HD], bf16)
            bv_sb = bpool.tile([1, HD], bf16)
            vT_sb = bpool.tile([128, T], bf16)

            nc.vector.memset(ones512[:], 1.0)
            nc.vector.memset(onescol[:], 1.0)
            nc.vector.memset(onesrow[:], 1.0)
            make_identity(nc, ident[:])

            # per-contraction-tile loads, in consumption order, so the first
            # projection matmuls start after ~640KB instead of ~13MB
            xT_r = xT.rearrange("(kt p) t -> p kt t", p=128)
            wqT_r = wqT.rearrange("(kt p) o -> p kt o", p=128)
            for kt in range(N_KT):
                nc.sync.dma_start(wqT_sb[:, kt, :], wqT_r[:, kt, :])
                nc.sync.dma_start(xT_sb[:, kt, :], xT_r[:, kt, :])
                if kt == 0:
                    nc.sync.dma_start(bq_sb[:], bq[:])
                    nc.sync.dma_start(ropeA_sb[:], ropeA[:])
                    nc.sync.dma_start(ropeB_sb[:], ropeB[:])
            nc.sync.dma_start(bk_sb[:], bk[:])
            nc.sync.dma_start(bv_sb[:], bv[:])
            nc.sync.dma_start(
                wkT_sb[:], wkT.rearrange("(kt p) o -> p kt o", p=128)
            )
            nc.sync.dma_start(
                wvT_sb[:], wvT.rearrange("(kt p) o -> p kt o", p=128)
            )
            nc.sync.dma_start(masks_sb[:], masks[:])
            nc.sync.dma_start(wpT_sb[:], wpT.rearrange("(h p) m -> p h m", p=128))

            def rope_epilogue(ps, dst, t0):
                """dst (bf16 SBUF) = rope(ps) using A/B tables; ps is fp32 psum
                [128, TC] holding a projected qT/kT chunk at positions t0:t0+TC."""
                A = ropeA_sb[:, t0 : t0 + TC]
                Bm = ropeB_sb[:, t0 : t0 + TC]
                tmp = tmp_pool.tile([128, TC], f32, tag="rope_tmp")
                tmp2 = tmp_pool.tile([128, TC], f32, tag="rope_tmp2")
                nc.vector.tensor_mul(tmp[0:64, :], ps[64:128, :], Bm[0:64, :])
                nc.vector.tensor_mul(tmp[64:128, :], ps[0:64, :], Bm[64:128, :])
                nc.vector.tensor_mul(tmp2[:], ps[:], A)
                nc.vector.tensor_add(dst, tmp2[:], tmp[:])

            # ---- phase B1: q projection (transposed) + rope ----
            with tc.tile_pool(name="qpsum", bufs=8, space="PSUM") as qpsum:
                for tci in range(N_TC):
                    t0 = tci * TC
                    for ot in range(N_REP):
                        ps = qpsum.tile([128, TC], f32)
                        for kt in range(N_KT):
                            nc.tensor.matmul(
                                ps[:],
                                lhsT=wqT_sb[:, kt, ot * 128 : (ot + 1) * 128],
                                rhs=xT_sb[:, kt, t0 : t0 + TC],
                                start=(kt == 0),
                                stop=False,
                            )
                        nc.tensor.matmul(
                            ps[:],
                            lhsT=bq_sb[:, ot * 128 : (ot + 1) * 128],
                            rhs=ones512[:],
                            start=False,
                            stop=True,
                        )
                        rope_epilogue(ps, qT_sb[:, ot, t0 : t0 + TC], t0)

            # ---- phase B2: k, v projections ----
            with (
                tc.tile_pool(name="kpsum", bufs=2, space="PSUM") as kpsum,
                tc.tile_pool(name="vpsum", bufs=2, space="PSUM") as vpsum,
                tc.tile_pool(name="tpsum", bufs=2, space="PSUM") as tpsum,
            ):
                for tci in range(N_TC):
                    t0 = tci * TC
                    psk = kpsum.tile([128, TC], f32)
                    for kt in range(N_KT):
                        nc.tensor.matmul(
                            psk[:],
                            lhsT=wkT_sb[:, kt, :],
                            rhs=xT_sb[:, kt, t0 : t0 + TC],
                            start=(kt == 0),
                            stop=False,
                        )
                    nc.tensor.matmul(
                        psk[:], lhsT=bk_sb[:], rhs=ones512[:], start=False, stop=True
                    )
                    rope_epilogue(psk, kT_sb[:, t0 : t0 + TC], t0)

                    psv = vpsum.tile([128, TC], f32)
                    for kt in range(N_KT):
                        nc.tensor.matmul(
                            psv[:],
                            lhsT=wvT_sb[:, kt, :],
                            rhs=xT_sb[:, kt, t0 : t0 + TC],
                            start=(kt == 0),
                            stop=False,
                        )
                    nc.tensor.matmul(
                        psv[:], lhsT=bv_sb[:], rhs=ones512[:], start=False, stop=True
                    )
                    nc.scalar.copy(vT_sb[:, t0 : t0 + TC], psv[:])

                    # transpose this chunk's vT -> v (natural [t, d]) via PE
                    for jt in range(4 * tci, 4 * tci + 4):
                        pst = tpsum.tile([128, 128], bf16)
                        nc.tensor.transpose(
                            pst[:], vT_sb[:, jt * 128 : (jt + 1) * 128], ident[:]
                        )
                        nc.scalar.copy(v_sb[:, jt, :], pst[:])

            bctx.close()

            # ---- phase C (attention) ----
            small = ctx.enter_context(tc.tile_pool(name="small", bufs=2))
            stage_pool = ctx.enter_context(tc.tile_pool(name="stage", bufs=4))
            epool = ctx.enter_context(tc.tile_pool(name="e", bufs=3))
            cpool = ctx.enter_context(tc.tile_pool(name="cphase", bufs=1))
            dci_pool = ctx.enter_context(tc.tile_pool(name="dci", bufs=2))
            Ln = mybir.ActivationFunctionType.Ln

            # unnormalized attention outputs, fp32
            yun_sb = cpool.tile([128, N_REP, T], f32)

            with (
                tc.tile_pool(name="spsum", bufs=2, space="PSUM") as spsum,
                tc.tile_pool(name="ypsum", bufs=3, space="PSUM") as ypsum,
                tc.tile_pool(name="dpsum", bufs=1, space="PSUM") as dpsum,
            ):

                def s_group(h, i0, g):
                    """Two score matmuls (key tiles 2g, 2g+1) into one 2-bank
                    psum tile, so exp can run 1024 wide."""
                    ps = spsum.tile([128, 2, TC], f32, tag="s")
                    for sub in range(2):
                        jt = 2 * g + sub
                        nc.tensor.matmul(
                            ps[:, sub, :],
                            lhsT=kT_sb[:, jt * 128 : (jt + 1) * 128],
                            rhs=qT_sb[:, h, i0 : i0 + TC],
                            start=True,
                            stop=True,
                        )
                    return ps

                def make_norm_thunks(ci, den_ci, i0):
                    rden_bf = dci_pool.tile([1, N_REP * TC], bf16, tag="rden")
                    lg = dci_pool.tile([1, N_REP * TC], f32, tag="lg")

                    def lnexp():
                        nc.scalar.activation(lg[:], den_ci[:], Ln)
                        nc.scalar.activation(rden_bf[:], lg[:], Exp, scale=-1.0)

                    def mk_h(h):
                        def norm_h():
                            ps_rb = spsum.tile([128, 2, TC], f32, tag="s")
                            nc.tensor.matmul(
                                ps_rb[:, 0, :],
                                lhsT=onesrow[:],
                                rhs=rden_bf[0:1, h * TC : (h + 1) * TC],
                                start=True,
                                stop=True,
                            )
                            rb_sb = stage_pool.tile([128, TC], f32, tag="rb_stage")
                            nc.vector.tensor_copy(rb_sb[:], ps_rb[:, 0, :])
                            nc.vector.tensor_mul(
                                yT_sb[:, h, i0 : i0 + TC],
                                yun_sb[:, h, i0 : i0 + TC],
                                rb_sb[:],
                            )

                        return norm_h

                    return [lnexp] + [mk_h(h) for h in range(N_REP)]

                norm_queue = []
                for ci in range(N_TC):
                    i0 = ci * TC
                    ng = 2 * (ci + 1)
                    den_ci = dci_pool.tile([1, N_REP * TC], f32, tag="den_ci")
                    for h in range(N_REP):
                        ps_y = ypsum.tile([128, TC], f32)
                        ps_den = dpsum.tile([1, TC], f32)
                        s_tiles = {0: s_group(h, i0, 0)}
                        if ng > 1:
                            s_tiles[1] = s_group(h, i0, 1)
                        # drain deferred normalization work from the previous chunk
                        for _ in range(2):
                            if norm_queue:
                                norm_queue.pop(0)()
                        for g in range(ng):
                            ps_s = s_tiles.pop(g)
                            e = epool.tile([128, 2, TC], bf16)
                            nc.scalar.activation(e[:], ps_s[:], Exp, scale=SCALE)
                            dg = g - 2 * ci
                            if dg >= 0:
                                nc.vector.tensor_mul(
                                    e[:], e[:], masks_sb[:, 2 * dg : 2 * dg + 2, :]
                                )
                            for sub in range(2):
                                jt = 2 * g + sub
                                nc.tensor.matmul(
                                    ps_y[:],
                                    lhsT=v_sb[:, jt, :],
                                    rhs=e[:, sub, :],
                                    start=(jt == 0),
                                    stop=(jt == 2 * ng - 1),
                                )
                                nc.tensor.matmul(
                                    ps_den[:],
                                    lhsT=onescol[:],
                                    rhs=e[:, sub, :],
                                    start=(jt == 0),
                                    stop=(jt == 2 * ng - 1),
                                )
                            if g + 2 < ng:
                                s_tiles[g + 2] = s_group(h, i0, g + 2)
                        # stage unnormalized outputs; free the psum banks fast
                        nc.vector.tensor_copy(
                            den_ci[0:1, h * TC : (h + 1) * TC], ps_den[:]
                        )
                        nc.vector.tensor_copy(yun_sb[:, h, i0 : i0 + TC], ps_y[:])
                    norm_queue.extend(make_norm_thunks(ci, den_ci, i0))
                while norm_queue:
                    norm_queue.pop(0)()

            # ---- phase D (output projection), PE-bound on its own ----
            with tc.tile_pool(name="opsum", bufs=4, space="PSUM") as opsum:
                for t_idx in range(T // 128):
                    o_sb = stage_pool.tile([128, N_TC, TC], f32, tag="o_stage")
                    for mc in range(N_TC):
                        ps_o = opsum.tile([128, TC], f32)
                        for h in range(N_REP):
                            nc.tensor.matmul(
                                ps_o[:],
                                lhsT=yT_sb[:, h, t_idx * 128 : (t_idx + 1) * 128],
                                rhs=wpT_sb[:, h, mc * TC : (mc + 1) * TC],
                                start=(h == 0),
                                stop=(h == N_REP - 1),
                            )
                        # split psum->sbuf copies across ACT and DVE
                        if mc % 2 == 0:
                            nc.scalar.copy(o_sb[:, mc, :], ps_o[:])
                        else:
                            nc.vector.tensor_copy(o_sb[:, mc, :], ps_o[:])
                    nc.sync.dma_start(
                        out[t_idx * 128 : (t_idx + 1) * 128, :], o_sb[:]
                    )

    nc.compile()
    return nc


def _get_compiled():
    global _compiled
    if _compiled is None:
        _compiled = _build()
    return _compiled


def kernel(x, cos, sin, wq, bq, wk, bk, wv, bv, wproj):
    from concourse.bass_utils import run_bass_kernel_spmd

    nc = _get_compiled()

    x = np.asarray(x, np.float32)
    cosT = np.asarray(cos, np.float32)[0, :, 0, :].T  # (64, T)
    sinT = np.asarray(sin, np.float32)[0, :, 0, :].T
    ropeA = np.ascontiguousarray(np.concatenate([cosT, cosT], 0))  # (128, T)
    ropeB = np.ascontiguousarray(np.concatenate([-sinT, sinT], 0))

    jj = np.arange(128, dtype=np.int64)[:, None, None]
    rr = np.arange(4, dtype=np.int64)[None, :, None]
    ii = np.arange(TC, dtype=np.int64)[None, None, :]
    masks = ((128 * rr + jj) <= ii).astype(BF16)  # (128, 4, 512)

    xT = [np.ascontiguousarray(x[b].T).astype(BF16) for b in range(2)]

    in_maps = []
    for c in range(8):
        b, g = divmod(c, 4)
        in_maps.append(
            {
                "xT": xT[b],
                "wqT": np.ascontiguousarray(
                    wq[g * O_G : (g + 1) * O_G].T
                ).astype(BF16),
                "wkT": np.ascontiguousarray(
                    wk[g * HD : (g + 1) * HD].T
                ).astype(BF16),
                "wvT": np.ascontiguousarray(
                    wv[g * HD : (g + 1) * HD].T
                ).astype(BF16),
                "wpT": np.ascontiguousarray(
                    wproj[:, g * O_G : (g + 1) * O_G].T
                ).astype(BF16),
                "bq": bq[None, g * O_G : (g + 1) * O_G].astype(BF16),
                "bk": bk[None, g * HD : (g + 1) * HD].astype(BF16),
                "bv": bv[None, g * HD : (g + 1) * HD].astype(BF16),
                "ropeA": ropeA,
                "ropeB": ropeB,
                "masks": masks,
            }
        )

    res = run_bass_kernel_spmd(nc, in_maps, core_ids=list(range(8)))
    parts = [res.results[c]["out"] for c in range(8)]
    out = np.stack(
        [
            parts[0] + parts[1] + parts[2] + parts[3],
            parts[4] + parts[5] + parts[6] + parts[7],
        ]
    ).astype(np.float32)
    return out
